# revision 18
# baseline (speedup 1.0000x reference)
import sys

if "/opt/trn_rl_repo" not in sys.path:
    sys.path.insert(0, "/opt/trn_rl_repo")

import numpy as np
import ml_dtypes
from contextlib import ExitStack

import concourse.bass as bass
from concourse import bacc
import concourse.tile as tile
from concourse import mybir
from concourse.bass_utils import run_bass_kernel_spmd

B, C, N, L = 16, 768, 1024, 16
SPC = 2
NCORES = 8
CB = 6
JB = 8
NBL = 2
JP = 4
F32 = mybir.dt.float32
F32R = mybir.dt.float32r
BF16 = mybir.dt.bfloat16
F8 = mybir.dt.float8e4
AF = mybir.ActivationFunctionType
ALU = mybir.AluOpType
DR = mybir.MatmulPerfMode.DoubleRow

NP_F8 = ml_dtypes.float8_e4m3
NP_BF = ml_dtypes.bfloat16

SK = 128.0
SX = 32.0
SS = 128.0
SH = 32.0
SW2 = 1024.0
SPW = 1024.0
CE = SS * SK * 1e-5
CV = 0.12 * SS / SX
LNSK = float(np.log(SK))
AUG = 32
XW = 776  # xs8 row: 768 x-cols + col 768 = SS*scol (d fold); even-aligned pad


def _sl(nb):
    return slice(nb * 512, (nb + 1) * 512)


def build_nc(reps=1):
    nc = bacc.Bacc(trn_type="TRN2")
    xo8_d = nc.declare_dram_parameter("xo8", [SPC, CB, 128, N], F8, isOutput=False)
    xt8_d = nc.declare_dram_parameter("xt8", [SPC, JB, 128, C], F8, isOutput=False)
    pw8_d = nc.declare_dram_parameter("pw8", [128, CB, L], F8, isOutput=False)
    pbc_d = nc.declare_dram_parameter("pbc", [L, 1], F32, isOutput=False)
    w1s_d = nc.declare_dram_parameter("w1s", [L, C], F32, isOutput=False)
    b1s_d = nc.declare_dram_parameter("b1s", [128, CB], F32, isOutput=False)
    w28_d = nc.declare_dram_parameter("w28", [128, CB, 16], F8, isOutput=False)
    b2c_d = nc.declare_dram_parameter("b2c", [1, 1], F32, isOutput=False)  # b2/2
    eye32_d = nc.declare_dram_parameter("eye32", [AUG + 1, 1], F32, isOutput=False)
    ones16_d = nc.declare_dram_parameter("ones16", [L, 1], F32, isOutput=False)
    onesrow_d = nc.declare_dram_parameter("onesrow", [1, N], F32, isOutput=False)
    out_d = nc.declare_dram_parameter("outT", [SPC, JB, 128, C], BF16, isOutput=True)

    with tile.TileContext(nc) as tc, ExitStack() as ctx:
        con = ctx.enter_context(tc.tile_pool(name="con", bufs=1))
        wrk = ctx.enter_context(tc.tile_pool(name="wrk", bufs=2))
        psp = ctx.enter_context(tc.tile_pool(name="psp", bufs=2, space="PSUM"))

        pw8 = con.tile([128, CB, L], F8)
        pbc = con.tile([L, 1], F32)
        w1f = con.tile([L, C], F32)
        b1s = con.tile([128, CB], F32)
        w28 = con.tile([128, CB, 16], F8)
        b2h = con.tile([1, 1], F32)
        eye32 = con.tile([AUG + 1, 1], F32)
        ones16 = con.tile([L, 1], F32)
        for t, d in ((pw8, pw8_d), (pbc, pbc_d), (w1f, w1s_d), (b1s, b1s_d),
                     (w28, w28_d), (b2h, b2c_d), (eye32, eye32_d), (ones16, ones16_d)):
            nc.sync.dma_start(t[:], d[:])
        lnsk_c = con.tile([128, 1], F32)
        nc.gpsimd.memset(lnsk_c[:], LNSK)
        ones16_r = con.tile([L, 1], F32R)
        nc.gpsimd.tensor_copy(ones16_r[:], ones16[:])
        w1r = con.tile([L, C], F32R)
        nc.gpsimd.tensor_copy(w1r[:], w1f[:])
        eye1 = eye32[0:1, :]

        def dma_in(s, si=99):
            st = {"si": si}
            st["xo8"] = wrk.tile([128, CB, N], F8, name="xo8", bufs=2)
            nc.sync.dma_start(st["xo8"][:], xo8_d[s].transpose((1, 0, 2)))
            st["xt8"] = wrk.tile([128, JB, C], F8, name="xt8", bufs=4)
            nc.sync.dma_start(st["xt8"][:], xt8_d[s].transpose((1, 0, 2)))
            return st

        # ---- front pieces (sample t) ----
        def f_z(st):
            # fps superimposes: z rows 0:16 (all cols), sq row 16, ntT cols 1016:1024
            fps = st["fps"] = psp.tile([128, N], F32, name="fps", tag="F", bufs=1)
            for nb in range(NBL):
                for t in range(CB // 2):
                    nc.tensor.matmul(fps[0:L, _sl(nb)], pw8[:, 2 * t:2 * t + 2, :],
                                     st["xo8"][:, 2 * t:2 * t + 2, _sl(nb)],
                                     start=(t == 0), stop=(t == CB // 2 - 1),
                                     perf_mode=DR)

        def f_zt(st, si):
            fps = st["fps"]
            # rows 0:16 = z ; zTR row32 = 1, row64 = -sq/2 ; zTL row32 = -sq/2,
            # row64 = 1 (aligned starts).  gram then carries -sq_i/2 - sq_j/2.
            zTR = st["zTR"] = wrk.tile([65, N], F32R, name="zTR", bufs=3)
            zTL = st["zTL"] = wrk.tile([65, N], F32R, name="zTL", bufs=2)
            if si < 3:
                nc.gpsimd.memset(zTR[:].bitcast(F32), 0.0)
                nc.gpsimd.memset(zTR[32:33, :].bitcast(F32), 1.0)
            if si < 2:
                nc.gpsimd.memset(zTL[:].bitcast(F32), 0.0)
                nc.gpsimd.memset(zTL[64:65, :].bitcast(F32), 1.0)
            nc.vector.tensor_scalar(zTR[0:L, :], fps[0:L, :], 1.0 / SPW, pbc[:],
                                    op0=ALU.mult, op1=ALU.add)
            nc.gpsimd.tensor_copy(zTL[0:L, :], zTR[0:L, :])

        def f_zsq(st):
            zsq = st["zsq"] = wrk.tile([L, N], F32R, name="zsq", bufs=2)
            nc.scalar.activation(zsq[:], st["fps"][0:L, :], AF.Square,
                                 bias=pbc[:], scale=1.0 / SPW)

        def f_sq(st):
            sqp = st["sqp"] = psp.tile([1, N], F32, name="sqp", tag="F", bufs=1)
            for nb in range(NBL):
                nc.tensor.matmul(sqp[0:1, _sl(nb)], ones16_r[:],
                                 st["zsq"][:, _sl(nb)], start=True, stop=True)

        def f_sqrow(st):
            nc.vector.tensor_scalar_mul(st["zTR"][64:65, :],
                                        st["sqp"][0:1, :], -0.5)
            nc.gpsimd.tensor_copy(st["zTL"][32:33, :], st["zTR"][64:65, :])

        def f_prep_k(st):
            st["k8"] = wrk.tile([128, JB, N], F8, name="k8", bufs=4)
            st["racc"] = wrk.tile([128, JB], F32, name="racc", bufs=3)

        def f_gram(st, j):
            g_ps = psp.tile([128, N], F32, name="g_ps", tag="G", bufs=2)
            for nb in range(NBL):
                nc.tensor.matmul(g_ps[:, _sl(nb)], st["zTL"][:, j * 128:(j + 1) * 128],
                                 st["zTR"][:, _sl(nb)], start=True, stop=True)
            nc.scalar.activation(st["k8"][:, j, :], g_ps[:], AF.Exp,
                                 bias=lnsk_c[:], scale=2.0,
                                 accum_out=st["racc"][:, j:j + 1])

        # ---- mid pieces (sample t-1) ----
        def m_prep(st):
            st["h8"] = wrk.tile([128, CB, N], F8, name="h8", bufs=2)
            st["first"] = False
            st["pips"] = [None, None]
            st["pi_row"] = wrk.tile([1, N], F32, name="pi_row", bufs=2)
            st["scolf"] = wrk.tile([128, JB], F32, name="scolf", bufs=2)
            st["xs8"] = wrk.tile([128, JB, XW], F8, name="xs8", bufs=3)
            if st.get("si", 99) < 3:
                nc.gpsimd.memset(st["xs8"][:, :, 769:776], 0.0)

        RELU_ENG = {(0, 0): "D", (1, 0): "A", (2, 0): "A", (3, 0): "D",
                    (4, 0): "D", (5, 0): "A", 
                    (0, 1): "D", (1, 1): "A", (2, 1): "D", (3, 1): "D",
                    (4, 1): "D", (5, 1): "D"}

        def m_h(st, cb, nb):
            h_ps = psp.tile([128, 512], F32, name="h_ps", tag="MB", bufs=2)
            nc.tensor.matmul(h_ps[:], w1r[:, cb * 128:(cb + 1) * 128],
                             st["zTR"][0:L, _sl(nb)], start=True, stop=True)
            e = RELU_ENG[(cb, nb)]
            if e == "A":
                nc.scalar.activation(st["h8"][:, cb, _sl(nb)], h_ps[:], AF.Relu,
                                     bias=b1s[:, cb:cb + 1], scale=1.0)
            elif e == "D":
                nc.vector.tensor_scalar(st["h8"][:, cb, _sl(nb)], h_ps[:],
                                        b1s[:, cb:cb + 1], 0.0,
                                        op0=ALU.add, op1=ALU.max)
            else:
                nc.gpsimd.tensor_scalar(st["h8"][:, cb, _sl(nb)], h_ps[:],
                                        b1s[:, cb:cb + 1], 0.0,
                                        op0=ALU.add, op1=ALU.max)

        def m_pi(st, nb):
            pips = psp.tile([128, 512], F32, name="pips", tag="MB", bufs=2)
            st["pips"][nb] = pips
            for t in range(CB // 2):
                nc.tensor.matmul(pips[0:16, :], w28[:, 2 * t:2 * t + 2, :],
                                 st["h8"][:, 2 * t:2 * t + 2, _sl(nb)],
                                 start=(t == 0), stop=(t == CB // 2 - 1),
                                 perf_mode=DR)
            # tanh(logit/2) (same ACT table as Exp; Sigmoid would reload table)
            nc.scalar.activation(st["pi_row"][:, _sl(nb)], pips[0:1, :], AF.Tanh,
                                 bias=b2h[:], scale=0.5 / (SH * SW2))

        def m_qr(st):
            qr = st["qr"] = wrk.tile([128, JB], F32, name="qr", bufs=2)
            nc.vector.reciprocal(qr[:], st["racc"][:])

        def m_scol(st, half):
            js = slice(half * 4, half * 4 + 4)
            pt_ps = psp.tile([128, 512], F32, name="pt_ps", tag="MB", bufs=2)
            for j in range(4):
                nc.tensor.transpose(pt_ps[:, j:j + 1],
                                    st["pi_row"][:, (half * 4 + j) * 128:
                                                 (half * 4 + j + 1) * 128], eye1)
            piB = wrk.tile([128, 4], F32, name="piB", bufs=4)
            nc.vector.tensor_scalar(piB[:], pt_ps[:, 0:4], 0.5 * SK, 0.5 * SK,
                                    op0=ALU.mult, op1=ALU.add)
            nc.vector.tensor_tensor(st["scolf"][:, js], piB[:], st["qr"][:, js],
                                    op=ALU.mult)

        def m_xs(st, j):
            nc.vector.tensor_scalar_mul(st["xs8"][:, j, 0:C], st["xt8"][:, j, :],
                                        st["scolf"][:, j:j + 1])

        def m_xcol(st, half):
            js = slice(half * 4, half * 4 + 4)
            nc.vector.tensor_scalar_mul(st["xs8"][:, js, 768:769],
                                        st["scolf"][:, js], SS)

        # ---- back pieces (sample t-2) ----
        def b_prep(st):
            st["vt"] = wrk.tile([128, JB], F32, name="vt", bufs=2)
            st["vcol"] = wrk.tile([128, JB], F32, name="vcol", bufs=2)
            st["outst"] = wrk.tile([128, JB, C], BF16, name="outst", bufs=2)

        O5_ENG = ["D", "A", "D", "D", "A", "D", "A", "D"]
        O2_ENG = ["A", "D", "D", "A", "D", "D", "A", "D"]

        def b_m(st, nblk):
            k8, xs8 = st["k8"], st["xs8"]
            m2 = psp.tile([128, 512], F32, name="m2", tag="MB", bufs=2)
            m1 = psp.tile([128, 512], F32, name="m1", tag="MB", bufs=2)
            for t in range(JP):
                nc.tensor.matmul(m2[:, 0:258], k8[:, 2 * t:2 * t + 2,
                                               nblk * 128:(nblk + 1) * 128],
                                 xs8[:, 2 * t:2 * t + 2, 512:770],
                                 start=(t == 0), stop=(t == JP - 1),
                                 perf_mode=DR)
            for t in range(JP):
                nc.tensor.matmul(m1[:], k8[:, 2 * t:2 * t + 2,
                                        nblk * 128:(nblk + 1) * 128],
                                 xs8[:, 2 * t:2 * t + 2, 0:512],
                                 start=(t == 0), stop=(t == JP - 1),
                                 perf_mode=DR)
            # vcol = CV / (d + CE) from the folded d column (psum col 256)
            nc.vector.tensor_scalar(st["vt"][:, nblk:nblk + 1], m2[:, 256:257],
                                    1.0 / CV, CE / CV, op0=ALU.mult, op1=ALU.add)
            nc.vector.reciprocal(st["vcol"][:, nblk:nblk + 1],
                                 st["vt"][:, nblk:nblk + 1])
            va = st["vcol"][:, nblk:nblk + 1]
            if O5_ENG[nblk] == "D":
                nc.vector.tensor_scalar_mul(st["outst"][:, nblk, 0:512], m1[:], va)
            else:
                nc.scalar.mul(st["outst"][:, nblk, 0:512], m1[:], va)
            if O2_ENG[nblk] == "D":
                nc.vector.tensor_scalar_mul(st["outst"][:, nblk, 512:768],
                                            m2[:, 0:256], va)
            else:
                nc.scalar.mul(st["outst"][:, nblk, 512:768], m2[:, 0:256], va)

        def b_out(st, s):
            nc.sync.dma_start(out_d[s].transpose((1, 0, 2)), st["outst"][:])

        # ---- slot emitter (4-stage, rotated grams): slot t runs z-chain(t+1)
        # + grams2-7/exps(t) + grams0-1/exps(t+1) + mid(t-1) + back(t-2) ----
        samples = [s for _ in range(reps) for s in range(SPC)]
        T = len(samples)
        states = {}

        states[0] = dma_in(samples[0], 0)
        if T > 1:
            states[1] = dma_in(samples[1], 1)
        f_z(states[0])
        f_zt(states[0], 0)
        f_zsq(states[0])
        f_sq(states[0])
        f_sqrow(states[0])
        f_prep_k(states[0])
        f_gram(states[0], 0)
        f_gram(states[0], 1)
        for t in range(T + 2):
            f = states.get(t) if t < T else None          # grams 2-7 + exps
            fe = states.get(t + 1) if t + 1 < T else None  # z-chain + grams 0-1
            m = states.get(t - 1) if 0 <= t - 1 < T else None  # mid sample
            b = states.get(t - 2) if t - 2 >= 0 else None  # back sample

            if f:
                f_gram(f, 2)
            if fe:
                f_z(fe)
            if t + 2 < T:
                states[t + 2] = dma_in(samples[t + 2], t + 2)
            if m:
                m_prep(m)
                m_h(m, 0, 0)
                m_h(m, 1, 0)
            if b:
                b_prep(b)
                b_m(b, 0)
            if fe:
                f_zt(fe, t + 1)
                f_zsq(fe)
            if f:
                f_gram(f, 3)
            if m:
                m_h(m, 2, 0)
                m_h(m, 3, 0)
            if b:
                b_m(b, 1)
            if fe:
                f_sq(fe)
                f_sqrow(fe)
            if f:
                f_gram(f, 4)
            if m:
                m_h(m, 4, 0)
                m_h(m, 5, 0)
                m_qr(m)
            if b:
                b_m(b, 2)
            if m:
                m_pi(m, 0)      # pips nb0 + tanh nb0
            if f:
                f_gram(f, 5)
            if m:
                m_scol(m, 0)    # pt j0-3 + piB/scolf half 0
                m_xs(m, 0)
                m_xs(m, 1)
                m_xs(m, 2)
                m_xs(m, 3)
                m_xcol(m, 0)
            if b:
                b_m(b, 3)
            if m:
                m_h(m, 0, 1)
                m_h(m, 1, 1)
            if f:
                f_gram(f, 6)
            if b:
                b_m(b, 4)
            if m:
                m_h(m, 2, 1)
                m_h(m, 3, 1)
                m_h(m, 4, 1)
                m_h(m, 5, 1)
            if f:
                f_gram(f, 7)
            if b:
                b_m(b, 5)
            if m:
                m_pi(m, 1)      # pips nb1 + tanh nb1
            if fe:
                f_prep_k(fe)
                f_gram(fe, 0)
            if b:
                b_m(b, 6)
            if m:
                m_scol(m, 1)    # pt j4-7 + piB/scolf half 1
                m_xs(m, 4)
                m_xs(m, 5)
                m_xs(m, 6)
                m_xs(m, 7)
                m_xcol(m, 1)
            if fe:
                f_gram(fe, 1)
            if b:
                b_m(b, 7)
                b_out(b, samples[t - 2])
                del states[t - 2]

    nc.compile()
    return nc


_NC_CACHE = {}


def _get_nc(reps=1):
    if reps not in _NC_CACHE:
        _NC_CACHE[reps] = build_nc(reps)
    return _NC_CACHE[reps]


def _f8(a):
    return np.ascontiguousarray(np.clip(a, -240, 240).astype(NP_F8))


_LAST_X = None


def make_in_maps(x, x_original, proj_w, proj_b, pi_w1, pi_b1, pi_w2, pi_b2):
    global _LAST_X
    xs = np.asarray(x, np.float32)[:, 0]
    _LAST_X = xs
    xos = np.asarray(x_original, np.float32)
    xsT = np.ascontiguousarray(xs.transpose(0, 2, 1))
    xo8 = _f8(xos).reshape(B, CB, 128, N)
    xt8 = _f8(xsT * SX).reshape(B, JB, 128, C)
    proj_w = np.asarray(proj_w, np.float32)
    pw8 = _f8((proj_w.T * SPW).reshape(CB, 128, L).transpose(1, 0, 2))
    pbc = np.ascontiguousarray(np.asarray(proj_b, np.float32).reshape(L, 1))
    w1s = np.ascontiguousarray(np.asarray(pi_w1, np.float32).T * SH)
    b1s = np.ascontiguousarray((np.asarray(pi_b1, np.float32) * SH)
                               .reshape(CB, 128).T)
    w28_col = np.ascontiguousarray((np.asarray(pi_w2, np.float32)[0] * SW2)
                                   .reshape(CB, 128).T).reshape(128, CB, 1)
    w28 = _f8(np.concatenate([w28_col, np.zeros((128, CB, 15), np.float32)], axis=2))
    b2c = np.asarray(pi_b2, np.float32).reshape(1, 1) * 0.5
    eye32 = np.zeros((AUG + 1, 1), np.float32)
    eye32[0, 0] = 1.0
    eye32[AUG, 0] = 1.0
    ones16 = np.ones((L, 1), np.float32)
    onesrow = np.ones((1, N), np.float32)
    in_maps = []
    for core in range(NCORES):
        sl = slice(SPC * core, SPC * (core + 1))
        in_maps.append({
            "xo8": np.ascontiguousarray(xo8[sl]),
            "xt8": np.ascontiguousarray(xt8[sl]),
            "pw8": pw8, "pbc": pbc, "w1s": w1s, "b1s": b1s,
            "w28": w28, "b2c": b2c, "eye32": eye32, "ones16": ones16,
            "onesrow": onesrow,
        })
    return in_maps


def assemble_output(results):
    dev = np.concatenate([np.asarray(results[i]["outT"]) for i in range(NCORES)],
                         axis=0)
    dev = dev.astype(np.float32).reshape(B, N, C).transpose(0, 2, 1)
    return np.ascontiguousarray(0.97 * _LAST_X + dev)


def run(inputs, trace=False):
    nc = _get_nc()
    in_maps = make_in_maps(**inputs)
    res = run_bass_kernel_spmd(nc, in_maps, list(range(NCORES)), trace=trace)
    return assemble_output(res.results), res


def kernel(**inputs):
    out, _ = run(inputs, trace=False)
    return out


# revision 20
# speedup vs baseline: 1.2814x; 1.2814x over previous
import sys

if "/opt/trn_rl_repo" not in sys.path:
    sys.path.insert(0, "/opt/trn_rl_repo")

import numpy as np
import ml_dtypes
from contextlib import ExitStack

import concourse.bass as bass
from concourse import bacc
import concourse.tile as tile
from concourse import mybir
from concourse.bass_utils import run_bass_kernel_spmd

B, C, N, L = 16, 768, 1024, 16
SPC = 2
NCORES = 8
CB = 6
JB = 8
NBL = 2
JP = 4
F32 = mybir.dt.float32
F32R = mybir.dt.float32r
BF16 = mybir.dt.bfloat16
F8 = mybir.dt.float8e4
AF = mybir.ActivationFunctionType
ALU = mybir.AluOpType
DR = mybir.MatmulPerfMode.DoubleRow

NP_F8 = ml_dtypes.float8_e4m3
NP_BF = ml_dtypes.bfloat16

SK = 128.0
SX = 32.0
SS = 128.0
SH = 32.0
SW2 = 1024.0
SPW = 1024.0
CE = SS * SK * 1e-5
CV = 0.12 * SS / SX
LNSK = float(np.log(SK))
AUG = 32
XW = 776


def _sl(nb):
    return slice(nb * 512, (nb + 1) * 512)


def build_nc(reps=1):
    nc = bacc.Bacc(trn_type="TRN2")
    xo8_d = nc.declare_dram_parameter("xo8", [SPC, CB, 128, N], F8, isOutput=False)
    xt8_d = nc.declare_dram_parameter("xt8", [SPC, JB, 128, C], F8, isOutput=False)
    pw8_d = nc.declare_dram_parameter("pw8", [128, CB, L], F8, isOutput=False)
    pbc_d = nc.declare_dram_parameter("pbc", [L, 1], F32, isOutput=False)
    w1s_d = nc.declare_dram_parameter("w1s", [L, C], F32, isOutput=False)
    b1s_d = nc.declare_dram_parameter("b1s", [128, CB], F32, isOutput=False)
    w28_d = nc.declare_dram_parameter("w28", [128, CB, 16], F8, isOutput=False)
    b2c_d = nc.declare_dram_parameter("b2c", [1, 1], F32, isOutput=False)  # holds b2/2
    eye32_d = nc.declare_dram_parameter("eye32", [AUG + 1, 1], F32, isOutput=False)
    ones16_d = nc.declare_dram_parameter("ones16", [L, 1], F32, isOutput=False)
    out_d = nc.declare_dram_parameter("outT", [SPC, JB, 128, C], BF16, isOutput=True)

    with tile.TileContext(nc) as tc, ExitStack() as ctx:
        con = ctx.enter_context(tc.tile_pool(name="con", bufs=1))
        wrk = ctx.enter_context(tc.tile_pool(name="wrk", bufs=2))
        psp = ctx.enter_context(tc.tile_pool(name="psp", bufs=2, space="PSUM"))

        pw8 = con.tile([128, CB, L], F8)
        pbc = con.tile([L, 1], F32)
        w1f = con.tile([L, C], F32)
        b1s = con.tile([128, CB], F32)
        w28 = con.tile([128, CB, 16], F8)
        b2h = con.tile([1, 1], F32)
        eye32 = con.tile([AUG + 1, 1], F32)
        ones16 = con.tile([L, 1], F32)
        for t, d in ((pw8, pw8_d), (pbc, pbc_d), (w1f, w1s_d), (b1s, b1s_d),
                     (w28, w28_d), (b2h, b2c_d), (eye32, eye32_d), (ones16, ones16_d)):
            nc.sync.dma_start(t[:], d[:])
        ones16_r = con.tile([L, 1], F32R)
        nc.gpsimd.tensor_copy(ones16_r[:], ones16[:])
        w1r = con.tile([L, C], F32R)
        nc.gpsimd.tensor_copy(w1r[:], w1f[:])
        eye1 = eye32[0:1, :]

        def front(s, si):
            st = {}
            xo8 = wrk.tile([128, CB, N], F8, name="xo8", bufs=2)
            nc.sync.dma_start(xo8[:], xo8_d[s].transpose((1, 0, 2)))
            xt8 = st["xt8"] = wrk.tile([128, JB, C], F8, name="xt8", bufs=3)
            nc.sync.dma_start(xt8[:], xt8_d[s].transpose((1, 0, 2)))

            zps = psp.tile([128, N], F32, name="zps", tag="F", bufs=2)
            for nb in range(NBL):
                for t in range(CB // 2):
                    nc.tensor.matmul(zps[0:L, _sl(nb)], pw8[:, 2 * t:2 * t + 2, :],
                                     xo8[:, 2 * t:2 * t + 2, _sl(nb)],
                                     start=(t == 0), stop=(t == CB // 2 - 1),
                                     perf_mode=DR)
            zTR = st["zTR"] = wrk.tile([AUG + 1, N], F32R, name="zTR", bufs=2)
            zTL = wrk.tile([AUG + 1, N], F32R, name="zTL", bufs=2)
            if si < 2:
                nc.gpsimd.memset(zTR[0:AUG, :].bitcast(F32), 0.0)
                nc.gpsimd.memset(zTL[0:AUG, :].bitcast(F32), 0.0)
                nc.gpsimd.memset(zTL[AUG:AUG + 1, :].bitcast(F32), 1.0)
            nc.vector.tensor_scalar(zTR[0:L, :], zps[0:L, :], 1.0 / SPW, pbc[:],
                                    op0=ALU.mult, op1=ALU.add)
            nc.vector.tensor_scalar(zTL[0:L, :], zps[0:L, :], 1.0 / SPW, pbc[:],
                                    op0=ALU.mult, op1=ALU.add)

            zsq = wrk.tile([L, N], F32R, name="zsq", bufs=2)
            nc.scalar.activation(zsq[:], zps[0:L, :], AF.Square,
                                 bias=pbc[:], scale=1.0 / SPW)
            sqps = psp.tile([128, N], F32, name="sqps", tag="F", bufs=2)
            for nb in range(NBL):
                nc.tensor.matmul(sqps[0:1, _sl(nb)], ones16_r[:],
                                 zsq[:, _sl(nb)], start=True, stop=True)
            nc.vector.tensor_scalar_mul(zTR[AUG:AUG + 1, :], sqps[0:1, :], -0.5)

            nt_ps = psp.tile([128, N], F32, name="nt_ps", tag="F", bufs=2)
            for j in range(JB):
                nc.tensor.transpose(nt_ps[:, j:j + 1],
                                    zTR[AUG:AUG + 1, j * 128:(j + 1) * 128]
                                    .bitcast(F32),
                                    eye32[AUG:AUG + 1, :])
            nsqln = wrk.tile([128, JB], F32, name="nsqln", bufs=2)
            nc.vector.tensor_scalar(nsqln[:], nt_ps[:, 0:JB], 2.0, LNSK,
                                    op0=ALU.mult, op1=ALU.add)

            k8 = st["k8"] = wrk.tile([128, JB, N], F8, name="k8", bufs=3)
            racc = st["racc"] = wrk.tile([128, JB], F32, name="racc", bufs=3)
            for j in range(JB):
                g_ps = psp.tile([128, N], F32, name="g_ps", tag="F", bufs=2)
                for nb in range(NBL):
                    nc.tensor.matmul(g_ps[:, _sl(nb)], zTL[:, j * 128:(j + 1) * 128],
                                     zTR[:, _sl(nb)], start=True, stop=True)
                nc.scalar.activation(k8[:, j, :], g_ps[:], AF.Exp,
                                     bias=nsqln[:, j:j + 1], scale=2.0,
                                     accum_out=racc[:, j:j + 1])
            return st

        def mid(st, si):
            zTR, xt8, racc = st["zTR"], st["xt8"], st["racc"]
            h8 = wrk.tile([128, CB, N], F8, name="h8", bufs=2)
            for cb in range(CB):
                for nb in range(NBL):
                    h_ps = psp.tile([128, 512], F32, name="h_ps", tag="Mid", bufs=2)
                    nc.tensor.matmul(h_ps[:], w1r[:, cb * 128:(cb + 1) * 128],
                                     zTR[0:L, _sl(nb)], start=True, stop=True)
                    if (cb + nb) % 2 == 0:
                        nc.vector.tensor_scalar(h8[:, cb, _sl(nb)], h_ps[:],
                                                b1s[:, cb:cb + 1],
                                                0.0, op0=ALU.add, op1=ALU.max)
                    else:
                        nc.scalar.activation(h8[:, cb, _sl(nb)], h_ps[:], AF.Relu,
                                             bias=b1s[:, cb:cb + 1], scale=1.0)
            pips = [psp.tile([128, 512], F32, name="pips", tag="Mid", bufs=2)
                    for _ in range(NBL)]
            for nb in range(NBL):
                for t in range(CB // 2):
                    nc.tensor.matmul(pips[nb][0:16, :], w28[:, 2 * t:2 * t + 2, :],
                                     h8[:, 2 * t:2 * t + 2, _sl(nb)],
                                     start=(t == 0), stop=(t == CB // 2 - 1),
                                     perf_mode=DR)
            pi_row = wrk.tile([1, N], F32, name="pi_row", bufs=2)
            for nb in range(NBL):
                # tanh(logit/2) (same ACT table as Exp; sigmoid would force a
                # 1.3us table reload every sample)
                nc.scalar.activation(pi_row[:, _sl(nb)], pips[nb][0:1, :], AF.Tanh,
                                     bias=b2h[:], scale=0.5 / (SH * SW2))

            pt_ps = psp.tile([128, 512], F32, name="pt_ps", tag="Mid", bufs=2)
            for j in range(JB):
                nc.tensor.transpose(pt_ps[:, j:j + 1],
                                    pi_row[:, j * 128:(j + 1) * 128], eye1)
            qr = wrk.tile([128, JB], F32, name="qr", bufs=2)
            nc.vector.reciprocal(qr[:], racc[:])
            piB = wrk.tile([128, JB], F32, name="piB", bufs=2)
            nc.vector.tensor_scalar(piB[:], pt_ps[:, 0:JB], 0.5 * SK, 0.5 * SK,
                                    op0=ALU.mult, op1=ALU.add)
            scolf = wrk.tile([128, JB], F32, name="scolf", bufs=2)
            nc.vector.tensor_tensor(scolf[:], piB[:], qr[:], op=ALU.mult)

            xs8 = st["xs8"] = wrk.tile([128, JB, XW], F8, name="xs8", bufs=3)
            if si < 3:
                nc.gpsimd.memset(xs8[:, :, 769:776], 0.0)
            nc.scalar.mul(xs8[:, :, 768:769], scolf[:], SS)
            for j in range(JB):
                if j % 2 == 0:
                    nc.scalar.mul(xs8[:, j, 0:C], xt8[:, j, :], scolf[:, j:j + 1])
                else:
                    nc.vector.tensor_scalar_mul(xs8[:, j, 0:C], xt8[:, j, :],
                                                scolf[:, j:j + 1])

        def back(st, s):
            k8, xs8 = st["k8"], st["xs8"]
            vt = wrk.tile([128, JB], F32, name="vt", bufs=2)
            vcol = wrk.tile([128, JB], F32, name="vcol", bufs=2)
            outst = wrk.tile([128, JB, C], BF16, name="outst", bufs=2)
            for nblk in range(JB):
                m2 = psp.tile([128, 512], F32, name="m2", tag="Back", bufs=2)
                m1 = psp.tile([128, 512], F32, name="m1", tag="Back", bufs=2)
                for t in range(JP):
                    nc.tensor.matmul(m2[:, 0:258], k8[:, 2 * t:2 * t + 2,
                                                   nblk * 128:(nblk + 1) * 128],
                                     xs8[:, 2 * t:2 * t + 2, 512:770],
                                     start=(t == 0), stop=(t == JP - 1),
                                     perf_mode=DR)
                for t in range(JP):
                    nc.tensor.matmul(m1[:], k8[:, 2 * t:2 * t + 2,
                                            nblk * 128:(nblk + 1) * 128],
                                     xs8[:, 2 * t:2 * t + 2, 0:512],
                                     start=(t == 0), stop=(t == JP - 1),
                                     perf_mode=DR)
                nc.vector.tensor_scalar(vt[:, nblk:nblk + 1], m2[:, 256:257],
                                        1.0 / CV, CE / CV, op0=ALU.mult, op1=ALU.add)
                nc.vector.reciprocal(vcol[:, nblk:nblk + 1], vt[:, nblk:nblk + 1])
                va = vcol[:, nblk:nblk + 1]
                if nblk % 2 == 0:
                    nc.scalar.mul(outst[:, nblk, 0:512], m1[:], va)
                    nc.scalar.mul(outst[:, nblk, 512:768], m2[:, 0:256], va)
                else:
                    nc.vector.tensor_scalar_mul(outst[:, nblk, 0:512], m1[:], va)
                    nc.vector.tensor_scalar_mul(outst[:, nblk, 512:768],
                                                m2[:, 0:256], va)
            nc.sync.dma_start(out_d[s].transpose((1, 0, 2)), outst[:])

        samples = [s for _ in range(reps) for s in range(SPC)]
        T = len(samples)
        state = {}
        for t in range(T + 2):
            if t - 2 >= 0:
                back(state.pop(t - 2), samples[t - 2])
            if 0 <= t - 1 < T:
                mid(state[t - 1], t - 1)
            if t < T:
                state[t] = front(samples[t], t)

    nc.compile()
    return nc


_NC_CACHE = {}


def _get_nc(reps=1):
    if reps not in _NC_CACHE:
        _NC_CACHE[reps] = build_nc(reps)
    return _NC_CACHE[reps]


def _f8(a):
    return np.ascontiguousarray(np.clip(a, -240, 240).astype(NP_F8))


_LAST_X = None


def make_in_maps(x, x_original, proj_w, proj_b, pi_w1, pi_b1, pi_w2, pi_b2):
    global _LAST_X
    xs = np.asarray(x, np.float32)[:, 0]
    _LAST_X = xs
    xos = np.asarray(x_original, np.float32)
    xsT = np.ascontiguousarray(xs.transpose(0, 2, 1))
    xo8 = _f8(xos).reshape(B, CB, 128, N)
    xt8 = _f8(xsT * SX).reshape(B, JB, 128, C)
    proj_w = np.asarray(proj_w, np.float32)
    pw8 = _f8((proj_w.T * SPW).reshape(CB, 128, L).transpose(1, 0, 2))
    pbc = np.ascontiguousarray(np.asarray(proj_b, np.float32).reshape(L, 1))
    w1s = np.ascontiguousarray(np.asarray(pi_w1, np.float32).T * SH)
    b1s = np.ascontiguousarray((np.asarray(pi_b1, np.float32) * SH)
                               .reshape(CB, 128).T)
    w28_col = np.ascontiguousarray((np.asarray(pi_w2, np.float32)[0] * SW2)
                                   .reshape(CB, 128).T).reshape(128, CB, 1)
    w28 = _f8(np.concatenate([w28_col, np.zeros((128, CB, 15), np.float32)], axis=2))
    b2c = np.asarray(pi_b2, np.float32).reshape(1, 1) * 0.5
    eye32 = np.zeros((AUG + 1, 1), np.float32)
    eye32[0, 0] = 1.0
    eye32[AUG, 0] = 1.0
    ones16 = np.ones((L, 1), np.float32)
    in_maps = []
    for core in range(NCORES):
        sl = slice(SPC * core, SPC * (core + 1))
        in_maps.append({
            "xo8": np.ascontiguousarray(xo8[sl]),
            "xt8": np.ascontiguousarray(xt8[sl]),
            "pw8": pw8, "pbc": pbc, "w1s": w1s, "b1s": b1s,
            "w28": w28, "b2c": b2c, "eye32": eye32, "ones16": ones16,
        })
    return in_maps


def assemble_output(results):
    dev = np.concatenate([np.asarray(results[i]["outT"]) for i in range(NCORES)],
                         axis=0)
    dev = dev.astype(np.float32).reshape(B, N, C).transpose(0, 2, 1)
    return np.ascontiguousarray(0.97 * _LAST_X + dev)


def run(inputs, trace=False):
    nc = _get_nc()
    in_maps = make_in_maps(**inputs)
    res = run_bass_kernel_spmd(nc, in_maps, list(range(NCORES)), trace=trace)
    return assemble_output(res.results), res


def kernel(**inputs):
    out, _ = run(inputs, trace=False)
    return out



# revision 22
# speedup vs baseline: 1.5778x; 1.2314x over previous
import sys

if "/opt/trn_rl_repo" not in sys.path:
    sys.path.insert(0, "/opt/trn_rl_repo")

import numpy as np
import ml_dtypes
from contextlib import ExitStack

import concourse.bass as bass
from concourse import bacc
import concourse.tile as tile
from concourse import mybir
from concourse.bass_utils import run_bass_kernel_spmd

B, C, N, L = 16, 768, 1024, 16
SPC = 2
NCORES = 8
CB = 6
JB = 8
NBL = 2
JP = 4
F32 = mybir.dt.float32
F32R = mybir.dt.float32r
BF16 = mybir.dt.bfloat16
F8 = mybir.dt.float8e4
AF = mybir.ActivationFunctionType
ALU = mybir.AluOpType
DR = mybir.MatmulPerfMode.DoubleRow

NP_F8 = ml_dtypes.float8_e4m3
NP_BF = ml_dtypes.bfloat16

SK = 128.0
SX = 32.0
SS = 128.0
SH = 32.0
SW2 = 1024.0
SPW = 1024.0
CE = SS * SK * 1e-5
CV = 0.12 * SS / SX
LNSK = float(np.log(SK))
AUG = 32


def _sl(nb):
    return slice(nb * 512, (nb + 1) * 512)


def build_nc(reps=1):
    nc = bacc.Bacc(trn_type="TRN2")
    xo8_d = nc.declare_dram_parameter("xo8", [SPC, CB, 128, N], F8, isOutput=False)
    xt8_d = nc.declare_dram_parameter("xt8", [SPC, JB, 128, C], F8, isOutput=False)
    pw8_d = nc.declare_dram_parameter("pw8", [128, CB, L], F8, isOutput=False)
    pbc_d = nc.declare_dram_parameter("pbc", [L, 1], F32, isOutput=False)
    w1s_d = nc.declare_dram_parameter("w1s", [L, C], F32, isOutput=False)
    b1s_d = nc.declare_dram_parameter("b1s", [128, CB], F32, isOutput=False)
    w28_d = nc.declare_dram_parameter("w28", [128, CB, 16], F8, isOutput=False)
    b2c_d = nc.declare_dram_parameter("b2c", [1, 1], F32, isOutput=False)  # holds b2/2
    eye32_d = nc.declare_dram_parameter("eye32", [AUG + 1, 1], F32, isOutput=False)
    ones16_d = nc.declare_dram_parameter("ones16", [L, 1], F32, isOutput=False)
    out_d = nc.declare_dram_parameter("outT", [SPC, JB, 128, C], BF16, isOutput=True)

    with tile.TileContext(nc) as tc, ExitStack() as ctx:
        con = ctx.enter_context(tc.tile_pool(name="con", bufs=1))
        wrk = ctx.enter_context(tc.tile_pool(name="wrk", bufs=2))
        psp = ctx.enter_context(tc.tile_pool(name="psp", bufs=2, space="PSUM"))

        pw8 = con.tile([128, CB, L], F8)
        pbc = con.tile([L, 1], F32)
        w1f = con.tile([L, C], F32)
        b1s = con.tile([128, CB], F32)
        w28 = con.tile([128, CB, 16], F8)
        b2h = con.tile([1, 1], F32)
        eye32 = con.tile([AUG + 1, 1], F32)
        ones16 = con.tile([L, 1], F32)
        for t, d in ((pw8, pw8_d), (pbc, pbc_d), (w1f, w1s_d), (b1s, b1s_d),
                     (w28, w28_d), (b2h, b2c_d), (eye32, eye32_d), (ones16, ones16_d)):
            nc.sync.dma_start(t[:], d[:])
        lnsk_c = con.tile([128, 1], F32)
        nc.gpsimd.memset(lnsk_c[:], LNSK)
        ones16_r = con.tile([L, 1], F32R)
        nc.gpsimd.tensor_copy(ones16_r[:], ones16[:])
        w1r = con.tile([L, C], F32R)
        nc.gpsimd.tensor_copy(w1r[:], w1f[:])
        eye1 = eye32[0:1, :]

        def front(s, si):
            st = {}
            xo8 = wrk.tile([128, CB, N], F8, name="xo8", bufs=2)
            nc.sync.dma_start(xo8[:], xo8_d[s].transpose((1, 0, 2)))
            xt8 = st["xt8"] = wrk.tile([128, JB, C], F8, name="xt8", bufs=3)
            nc.sync.dma_start(xt8[:], xt8_d[s].transpose((1, 0, 2)))

            zps = psp.tile([128, N], F32, name="zps", tag="F", bufs=2)
            for nb in range(NBL):
                for t in range(CB // 2):
                    nc.tensor.matmul(zps[0:L, _sl(nb)], pw8[:, 2 * t:2 * t + 2, :],
                                     xo8[:, 2 * t:2 * t + 2, _sl(nb)],
                                     start=(t == 0), stop=(t == CB // 2 - 1),
                                     perf_mode=DR)
            # zTR rows: 0:16 z, 32 = 1, 64 = -sq/2 ; zTL rows: 0:16 z,
            # 32 = -sq/2, 64 = 1.  gram then carries -sq_i/2 - sq_j/2 itself.
            zTR = st["zTR"] = wrk.tile([65, N], F32R, name="zTR", bufs=2)
            zTL = st["zTL"] = wrk.tile([65, N], F32R, name="zTL", bufs=2)
            if si < 2:
                nc.gpsimd.memset(zTR[:].bitcast(F32), 0.0)
                nc.gpsimd.memset(zTR[32:33, :].bitcast(F32), 1.0)
                nc.gpsimd.memset(zTL[:].bitcast(F32), 0.0)
                nc.gpsimd.memset(zTL[64:65, :].bitcast(F32), 1.0)
            nc.vector.tensor_scalar(zTR[0:L, :], zps[0:L, :], 1.0 / SPW, pbc[:],
                                    op0=ALU.mult, op1=ALU.add)
            nc.gpsimd.tensor_copy(zTL[0:L, :], zTR[0:L, :])

            zsq = wrk.tile([L, N], F32R, name="zsq", bufs=2)
            nc.scalar.activation(zsq[:], zps[0:L, :], AF.Square,
                                 bias=pbc[:], scale=1.0 / SPW)
            sqps = psp.tile([128, N], F32, name="sqps", tag="F", bufs=2)
            for nb in range(NBL):
                nc.tensor.matmul(sqps[0:1, _sl(nb)], ones16_r[:],
                                 zsq[:, _sl(nb)], start=True, stop=True)
            nc.vector.tensor_scalar_mul(zTR[64:65, :], sqps[0:1, :], -0.5)
            nc.gpsimd.tensor_copy(zTL[32:33, :], zTR[64:65, :])

            k8 = st["k8"] = wrk.tile([128, JB, N], F8, name="k8", bufs=3)
            racc = st["racc"] = wrk.tile([128, JB], F32, name="racc", bufs=3)
            for j in range(JB):
                g_ps = psp.tile([128, N], F32, name="g_ps", tag="F", bufs=2)
                for nb in range(NBL):
                    nc.tensor.matmul(g_ps[:, _sl(nb)], zTL[:, j * 128:(j + 1) * 128],
                                     zTR[:, _sl(nb)], start=True, stop=True)
                nc.scalar.activation(k8[:, j, :], g_ps[:], AF.Exp,
                                     bias=lnsk_c[:], scale=2.0,
                                     accum_out=racc[:, j:j + 1])
            return st

        def mid(st, si):
            zTR, xt8, racc = st["zTR"], st["xt8"], st["racc"]
            h8 = wrk.tile([128, CB, N], F8, name="h8", bufs=2)
            for cb in range(CB):
                for nb in range(NBL):
                    h_ps = psp.tile([128, 512], F32, name="h_ps", tag="Mid", bufs=2)
                    nc.tensor.matmul(h_ps[:], w1r[:, cb * 128:(cb + 1) * 128],
                                     zTR[0:L, _sl(nb)], start=True, stop=True)
                    if (cb + nb) % 2 == 0 or cb >= 4:
                        nc.vector.tensor_scalar(h8[:, cb, _sl(nb)], h_ps[:],
                                                b1s[:, cb:cb + 1],
                                                0.0, op0=ALU.add, op1=ALU.max)
                    else:
                        nc.scalar.activation(h8[:, cb, _sl(nb)], h_ps[:], AF.Relu,
                                             bias=b1s[:, cb:cb + 1], scale=1.0)
            pips = [psp.tile([128, 512], F32, name="pips", tag="Mid", bufs=2)
                    for _ in range(NBL)]
            for nb in range(NBL):
                for t in range(CB // 2):
                    nc.tensor.matmul(pips[nb][0:16, :], w28[:, 2 * t:2 * t + 2, :],
                                     h8[:, 2 * t:2 * t + 2, _sl(nb)],
                                     start=(t == 0), stop=(t == CB // 2 - 1),
                                     perf_mode=DR)
            pi_row = wrk.tile([1, N], F32, name="pi_row", bufs=2)
            for nb in range(NBL):
                # tanh(logit/2) (same ACT table as Exp; sigmoid would force a
                # 1.3us table reload every sample)
                nc.scalar.activation(pi_row[:, _sl(nb)], pips[nb][0:1, :], AF.Tanh,
                                     bias=b2h[:], scale=0.5 / (SH * SW2))

            pt_ps = psp.tile([128, 512], F32, name="pt_ps", tag="Mid", bufs=2)
            for j in range(JB):
                nc.tensor.transpose(pt_ps[:, j:j + 1],
                                    pi_row[:, j * 128:(j + 1) * 128], eye1)
            qr = wrk.tile([128, JB], F32, name="qr", bufs=2)
            nc.vector.reciprocal(qr[:], racc[:])
            piB = wrk.tile([128, JB], F32, name="piB", bufs=2)
            nc.vector.tensor_scalar(piB[:], pt_ps[:, 0:JB], 0.5 * SK, 0.5 * SK,
                                    op0=ALU.mult, op1=ALU.add)
            scolf = wrk.tile([128, JB], F32, name="scolf", bufs=2)
            nc.vector.tensor_tensor(scolf[:], piB[:], qr[:], op=ALU.mult)
            s8 = st["s8"] = wrk.tile([128, JB, 16], F8, name="s8", bufs=3)
            if si < 3:
                nc.gpsimd.memset(s8[:], 0.0)
            nc.scalar.mul(s8[:, :, 0:1], scolf[:], SS)

            xs8 = st["xs8"] = wrk.tile([128, JB, C], F8, name="xs8", bufs=3)
            for j in range(JB):
                nc.vector.tensor_scalar_mul(xs8[:, j, :], xt8[:, j, :],
                                            scolf[:, j:j + 1])

        def back(st, s):
            k8, s8, xs8 = st["k8"], st["s8"], st["xs8"]
            dps = [psp.tile([128, 512], F32, name="dps", tag="Back", bufs=2)
                   for _ in range(NBL)]
            for nb in range(NBL):
                for t in range(JP):
                    nc.tensor.matmul(dps[nb][0:16, :], s8[:, 2 * t:2 * t + 2, :],
                                     k8[:, 2 * t:2 * t + 2, _sl(nb)],
                                     start=(t == 0), stop=(t == JP - 1),
                                     perf_mode=DR)
            d_row = wrk.tile([1, N], F32, name="d_row", bufs=2)
            for nb in range(NBL):
                nc.vector.tensor_copy(d_row[:, _sl(nb)], dps[nb][0:1, :])

            dt_ps = psp.tile([128, 512], F32, name="dt_ps", tag="Back", bufs=2)
            for j in range(JB):
                nc.tensor.transpose(dt_ps[:, j:j + 1],
                                    d_row[:, j * 128:(j + 1) * 128], eye1)
            dtc = wrk.tile([128, JB], F32, name="dtc", bufs=2)
            nc.vector.tensor_scalar_add(dtc[:], dt_ps[:, 0:JB], CE)
            dtr = wrk.tile([128, JB], F32, name="dtr", bufs=2)
            nc.vector.reciprocal(dtr[:], dtc[:])
            vcol = wrk.tile([128, JB], F32, name="vcol", bufs=2)
            nc.vector.tensor_scalar_mul(vcol[:], dtr[:], CV)

            outst = wrk.tile([128, JB, C], BF16, name="outst", bufs=2)
            for nblk in range(JB):
                m1 = psp.tile([128, 512], F32, name="m1", tag="Back", bufs=2)
                m2 = psp.tile([128, 512], F32, name="m2", tag="Back", bufs=2)
                for t in range(JP):
                    nc.tensor.matmul(m1[:], k8[:, 2 * t:2 * t + 2,
                                            nblk * 128:(nblk + 1) * 128],
                                     xs8[:, 2 * t:2 * t + 2, 0:512],
                                     start=(t == 0), stop=(t == JP - 1),
                                     perf_mode=DR)
                for t in range(JP):
                    nc.tensor.matmul(m2[:, 0:256], k8[:, 2 * t:2 * t + 2,
                                                   nblk * 128:(nblk + 1) * 128],
                                     xs8[:, 2 * t:2 * t + 2, 512:768],
                                     start=(t == 0), stop=(t == JP - 1),
                                     perf_mode=DR)
                if nblk % 2 == 0:
                    nc.scalar.mul(outst[:, nblk, 0:512], m1[:], vcol[:, nblk:nblk + 1])
                    nc.scalar.mul(outst[:, nblk, 512:768], m2[:, 0:256],
                                  vcol[:, nblk:nblk + 1])
                else:
                    nc.vector.tensor_scalar_mul(outst[:, nblk, 0:512], m1[:],
                                                vcol[:, nblk:nblk + 1])
                    nc.vector.tensor_scalar_mul(outst[:, nblk, 512:768], m2[:, 0:256],
                                                vcol[:, nblk:nblk + 1])
            nc.sync.dma_start(out_d[s].transpose((1, 0, 2)), outst[:])

        samples = [s for _ in range(reps) for s in range(SPC)]
        T = len(samples)
        state = {}
        for t in range(T + 2):
            if t - 2 >= 0:
                back(state.pop(t - 2), samples[t - 2])
            if 0 <= t - 1 < T:
                mid(state[t - 1], t - 1)
            if t < T:
                state[t] = front(samples[t], t)

    nc.compile()
    return nc


_NC_CACHE = {}


def _get_nc(reps=1):
    if reps not in _NC_CACHE:
        _NC_CACHE[reps] = build_nc(reps)
    return _NC_CACHE[reps]


def _f8(a):
    return np.ascontiguousarray(np.clip(a, -240, 240).astype(NP_F8))


_LAST_X = None


def make_in_maps(x, x_original, proj_w, proj_b, pi_w1, pi_b1, pi_w2, pi_b2):
    global _LAST_X
    xs = np.asarray(x, np.float32)[:, 0]
    _LAST_X = xs
    xos = np.asarray(x_original, np.float32)
    xsT = np.ascontiguousarray(xs.transpose(0, 2, 1))
    xo8 = _f8(xos).reshape(B, CB, 128, N)
    xt8 = _f8(xsT * SX).reshape(B, JB, 128, C)
    proj_w = np.asarray(proj_w, np.float32)
    pw8 = _f8((proj_w.T * SPW).reshape(CB, 128, L).transpose(1, 0, 2))
    pbc = np.ascontiguousarray(np.asarray(proj_b, np.float32).reshape(L, 1))
    w1s = np.ascontiguousarray(np.asarray(pi_w1, np.float32).T * SH)
    b1s = np.ascontiguousarray((np.asarray(pi_b1, np.float32) * SH)
                               .reshape(CB, 128).T)
    w28_col = np.ascontiguousarray((np.asarray(pi_w2, np.float32)[0] * SW2)
                                   .reshape(CB, 128).T).reshape(128, CB, 1)
    w28 = _f8(np.concatenate([w28_col, np.zeros((128, CB, 15), np.float32)], axis=2))
    b2c = np.asarray(pi_b2, np.float32).reshape(1, 1) * 0.5
    eye32 = np.zeros((AUG + 1, 1), np.float32)
    eye32[0, 0] = 1.0
    eye32[AUG, 0] = 1.0
    ones16 = np.ones((L, 1), np.float32)
    in_maps = []
    for core in range(NCORES):
        sl = slice(SPC * core, SPC * (core + 1))
        in_maps.append({
            "xo8": np.ascontiguousarray(xo8[sl]),
            "xt8": np.ascontiguousarray(xt8[sl]),
            "pw8": pw8, "pbc": pbc, "w1s": w1s, "b1s": b1s,
            "w28": w28, "b2c": b2c, "eye32": eye32, "ones16": ones16,
        })
    return in_maps


def assemble_output(results):
    dev = np.concatenate([np.asarray(results[i]["outT"]) for i in range(NCORES)],
                         axis=0)
    dev = dev.astype(np.float32).reshape(B, N, C).transpose(0, 2, 1)
    return np.ascontiguousarray(0.97 * _LAST_X + dev)


def run(inputs, trace=False):
    nc = _get_nc()
    in_maps = make_in_maps(**inputs)
    res = run_bass_kernel_spmd(nc, in_maps, list(range(NCORES)), trace=trace)
    return assemble_output(res.results), res


def kernel(**inputs):
    out, _ = run(inputs, trace=False)
    return out

